# revision 11
# baseline (speedup 1.0000x reference)
"""Trainium2 Bass kernel for nn_BertFreezeSegmentor (BiLSTM + stack-decoder).

Structure (validated against the reference, rel err ~1.6e-3):
  - Gold actions are in {0,1}, so the decoder "stacks" collapse into
    conditional carries, applied as data-driven hold/update masks.
  - All x-projections are hoisted out of the recurrences into GEMMs; each
    recurrence step only needs its h @ Whh.T matmul (LDWEIGHTS-bound).
  - Phase 1: core pairs (p, p+4) split each 16-example batch slice: low
    core runs the FWD scan, high core the BWD scan on host-reversed input
    (B=16 each). Histories merge via a pairwise AllGather.
  - Phase 2 (pipelined decode): the low core runs the subword chain for
    the pair's 16 examples; per 32-step block its (h1,c1) outputs are
    handed to the high core via a pairwise AllGather, where the word-Wih
    GEMM and the word chain run one block behind. The classifier GEMMs
    ride along per block (cx on low, wcls on high, one shared output).
  - Per-step gate adds are split per gate group so the i/f/g activations
    and the cell update overlap the o-gate m-tile matmuls.
  - The program is SPMD-uniform: all role divergence (weights, input
    order, carry masks, blend selects) is per-core input data.
"""

import numpy as np
import ml_dtypes

import concourse.bass as bass
import concourse.tile as tile
from concourse import bacc, mybir
from concourse.bass_utils import run_bass_kernel_spmd

BF16 = ml_dtypes.bfloat16
F8E3 = ml_dtypes.float8_e3m4
DT_BF = mybir.dt.bfloat16
DT_F8 = mybir.dt.float8e3
DT_F32 = mybir.dt.float32
AF = mybir.ActivationFunctionType

S, H, B16, B8 = 256, 768, 16, 8
NCORES = 8
QSCALE = 64.0            # fp8 weight scale: w_fp8 = w * QSCALE (e3m4)
INVQ = 1.0 / QSCALE      # folded into the gate activations' scale


def build_program6(num_devices=8, unroll=32):
    CH = H // 128            # 6
    GM = 4 * H // 128        # 24
    C2 = 2 * H // 128        # 12
    NC16 = S * B16           # 4096
    NC8 = S * B8             # 2048
    NB = 512
    NBLK16 = NC16 // NB      # 8
    NBLK8 = NC8 // NB        # 4

    nc = bacc.Bacc("TRN2", target_bir_lowering=False, debug=False,
                   enable_asserts=False, num_devices=num_devices)

    def inp(name, shape, dt):
        return nc.dram_tensor(name, shape, dt, kind="ExternalInput").ap()

    def scratch(name, shape, dt, **kw):
        return nc.dram_tensor(name, shape, dt, kind="Internal", **kw).ap()

    def outp(name, shape, dt):
        return nc.dram_tensor(name, shape, dt, kind="ExternalOutput").ap()

    # ---- inputs (per-core role data) ----
    xT16 = inp("xT16", [128, CH, NC16], DT_BF)       # straight / reversed
    wih = inp("wih", [128, CH, 4 * H], DT_BF)        # fwd / bwd (pre-scaled)
    whh = inp("whh", [128, CH, 4 * H], DT_F8)        # fp8e3, w*QSCALE
    biasg = inp("biasg", [1, 4 * H], DT_BF)
    selA = inp("selA", [128, 1], DT_F32)             # 1 on low cores, 0 high
    selB = inp("selB", [128, 1], DT_F32)             # complement
    g1w = inp("g1w", [128, C2, 4 * H], DT_BF)       # swih / wwih (pre-scaled)
    g1bias = inp("g1bias", [1, 4 * H], DT_BF)        # sbias / wbias (pre-scaled)
    chw = inp("chw", [128, CH, 4 * H], DT_F8)        # fp8e3, w*QSCALE
    auxw = inp("auxw", [128, C2, 2], DT_BF)          # cls2T / [cls1T;0]
    NIT = 9                                          # pipeline iterations
    SB = 32 * B16                                    # block cols (512)
    maskA = inp("maskA", [128, CH, NIT * SB], DT_BF)
    maskB = inp("maskB", [128, CH, NIT * SB], DT_BF)

    # ---- scratch ----
    XS = scratch("XS", [128, GM, NC16], DT_BF)
    histS = scratch("histS", [128, CH, NC16], DT_BF)
    histR = scratch("histR", [128, CH, NC16], DT_BF)
    canonT = scratch("canonT", [128, CH, NC16], DT_BF)
    AGOUT = scratch("AGOUT", [2, 128, CH, S, B16], DT_BF)
    AGB = [scratch(f"AGB{k}", [128, C2, SB], DT_BF) for k in range(8)]
    AGO = [scratch(f"AGO{k}", [2, 128, C2, SB], DT_BF) for k in range(8)]

    # ---- outputs ----
    aux_t = outp("aux_t", [2, NIT * SB], DT_F32)

    RG = [[p, p + 4] for p in range(4)]

    with tile.TileContext(nc) as tc:

        _dma_rr = [0]

        def dma_eng():
            _dma_rr[0] += 1
            return nc.sync if _dma_rr[0] % 2 else nc.gpsimd

        def load_w(pool, src, tag):
            t = pool.tile(list(src.shape), src.dtype, tag=tag)
            if len(src.shape) == 3 and src.shape[1] > 1:
                for k in range(src.shape[1]):
                    dma_eng().dma_start(t[:, k, :], src[:, k, :])
            else:
                dma_eng().dma_start(t[:], src[:])
            return t

        # ============ Phase A2: XS = wih @ x + bias  (B=16) ============
        with tc.tile_pool(name="wA", bufs=1) as wp, \
             tc.tile_pool(name="gA", bufs=3) as pool, \
             tc.tile_pool(name="gA_ps", bufs=2,
                          space=bass.MemorySpace.PSUM) as psp:
            ones = wp.tile([1, NB], DT_BF, tag="ones")
            nc.vector.memset(ones[:], 1.0)
            wih_sb = load_w(wp, wih, "wih_sb")
            bia_sb = load_w(wp, biasg, "bia_sb")
            for nb in range(NBLK16):
                mv = pool.tile([128, CH, NB], DT_BF, tag="mvA")
                for k in range(CH):
                    dma_eng().dma_start(mv[:, k, :],
                                        xT16[:, k, bass.ts(nb, NB)])
                for m in range(GM):
                    ps = psp.tile([128, NB], DT_F32, tag="ps")
                    for k in range(CH):
                        nc.tensor.matmul(
                            ps[:], wih_sb[:, k, bass.ts(m, 128)], mv[:, k, :],
                            start=(k == 0), stop=False)
                    nc.tensor.matmul(ps[:], bia_sb[:, bass.ts(m, 128)],
                                     ones[:], start=False, stop=True)
                    ot = pool.tile([128, NB], DT_BF, tag="gout")
                    nc.vector.tensor_copy(ot[:], ps[:])
                    dma_eng().dma_start(XS[:, m, bass.ts(nb, NB)], ot[:])

        # ============ Phase B2: one scan, B=16, dual history write ========
        with tc.tile_pool(name="w_scan", bufs=1) as wp, \
             tc.tile_pool(name="scan", bufs=3) as sp, \
             tc.tile_pool(name="scan_ps", bufs=2,
                          space=bass.MemorySpace.PSUM) as pp:
            whh_sb = load_w(wp, whh, "whh_sb")
            c0 = wp.tile([128, CH, B16], DT_F32, tag="c0")
            c1 = wp.tile([128, CH, B16], DT_F32, tag="c1")
            h0 = wp.tile([128, CH, B16], DT_BF, tag="h0")
            h1 = wp.tile([128, CH, B16], DT_BF, tag="h1")
            nc.vector.memset(c0[:], 0.0)
            nc.vector.memset(h0[:], 0.0)
            cc, hh = [c0, c1], [h0, h1]

            def blk(iv0, cnt):
                xf = sp.tile([128, GM, unroll * B16], DT_BF, tag="xf")
                nc.sync.dma_start(
                    xf[:, :, 0:cnt * B16],
                    XS[:, :, bass.ds(iv0 * B16, cnt * B16)])
                for i in range(cnt):
                    t = iv0 + i
                    cprev, cnew = cc[i % 2], cc[(i + 1) % 2]
                    hprev, hnext = hh[i % 2], hh[(i + 1) % 2]
                    psA = pp.tile([128, 2 * CH, B16], DT_F32, tag="gA")
                    psG = pp.tile([128, CH, B16], DT_F32, tag="gG")
                    psO = pp.tile([128, CH, B16], DT_F32, tag="gO")
                    for m in range(GM):
                        if m < 2 * CH:
                            dst = psA[:, m, :]
                        elif m < 3 * CH:
                            dst = psG[:, m - 2 * CH, :]
                        else:
                            dst = psO[:, m - 3 * CH, :]
                        for k in range(CH):
                            nc.tensor.matmul(
                                dst, whh_sb[:, k, bass.ts(m, 128)],
                                hprev[:, k, :],
                                start=(k == 0), stop=(k == CH - 1))
                    g = sp.tile([128, GM, B16], DT_F32, tag="gs")
                    xfs = xf[:, :, i * B16:(i + 1) * B16]
                    nc.vector.tensor_add(
                        g[:, 0:2 * CH, :], psA[:], xfs[:, 0:2 * CH, :])
                    sif = sp.tile([128, 2 * CH, B16], DT_F32, tag="sif")
                    nc.scalar.activation(sif[:], g[:, 0:2 * CH, :],
                                         AF.Sigmoid, scale=INVQ)
                    nc.vector.tensor_add(
                        g[:, 2 * CH:3 * CH, :], psG[:],
                        xfs[:, 2 * CH:3 * CH, :])
                    nc.vector.tensor_add(
                        g[:, 3 * CH:4 * CH, :], psO[:],
                        xfs[:, 3 * CH:4 * CH, :])
                    tg = sp.tile([128, CH, B16], DT_F32, tag="tg")
                    nc.scalar.activation(tg[:], g[:, 2 * CH:3 * CH, :],
                                         AF.Tanh, scale=INVQ)
                    so = sp.tile([128, CH, B16], DT_F32, tag="so")
                    nc.scalar.activation(so[:], g[:, 3 * CH:4 * CH, :],
                                         AF.Sigmoid, scale=INVQ)
                    t1 = sp.tile([128, CH, B16], DT_F32, tag="t1")
                    nc.vector.tensor_mul(t1[:], sif[:, CH:2 * CH, :], cprev[:])
                    t2 = sp.tile([128, CH, B16], DT_F32, tag="t2")
                    nc.vector.tensor_mul(t2[:], sif[:, 0:CH, :], tg[:])
                    nc.vector.tensor_add(cnew[:], t1[:], t2[:])
                    th = sp.tile([128, CH, B16], DT_F32, tag="th")
                    nc.scalar.activation(th[:], cnew[:], AF.Tanh)
                    nc.vector.tensor_mul(hnext[:], so[:], th[:])
                    dma_eng().dma_start(
                        histS[:, :, bass.ds(t * B16, B16)], hnext[:])
                    dma_eng().dma_start(
                        histR[:, :, bass.ds((S - 1 - t) * B16, B16)],
                        hnext[:])

            tc.For_i_unrolled_general(0, S, 1, blk, max_unroll=unroll,
                                      hint_engines=(mybir.EngineType.PE,))

        # ============ blend canonT + pairwise AllGather ============
        with tc.tile_pool(name="blend", bufs=2) as bp:
            sA = bp.tile([128, 1], DT_F32, tag="sA")
            sB = bp.tile([128, 1], DT_F32, tag="sB")
            nc.sync.dma_start(sA[:], selA[:])
            nc.sync.dma_start(sB[:], selB[:])
            for k in range(CH):
                hs = bp.tile([128, NC16], DT_BF, tag="hs")
                hr = bp.tile([128, NC16], DT_BF, tag="hr")
                nc.sync.dma_start(hs[:], histS[:, k, :])
                nc.gpsimd.dma_start(hr[:], histR[:, k, :])
                a1 = bp.tile([128, NC16], DT_F32, tag="a1")
                nc.scalar.activation(a1[:], hs[:], AF.Copy, scale=sA[:])
                a2 = bp.tile([128, NC16], DT_F32, tag="a2")
                nc.scalar.activation(a2[:], hr[:], AF.Copy, scale=sB[:])
                cn = bp.tile([128, NC16], DT_BF, tag="cn")
                nc.vector.tensor_add(cn[:], a1[:], a2[:])
                nc.sync.dma_start(canonT[:, k, :], cn[:])

        nc.gpsimd.collective_compute(
            "AllGather", mybir.AluOpType.bypass,
            replica_groups=RG,
            ins=[canonT[:]],
            outs=[AGOUT[:]],
        )

        # ============ pipelined decode: sub (low) / word (high) ============
        # 9 iterations; low runs subword blocks 0-7 at iters 0-7, high runs
        # word blocks 0-7 at iters 1-8 (one-block lag). Handoff of h1/c1 is
        # a per-block pairwise AllGather; high's GEMM1 (wwih) consumes the
        # previous iteration's AG output, low's GEMM1 (swih) consumes the
        # lstm_out block from the big AG. All role divergence is input data.
        with tc.tile_pool(name="wD", bufs=1) as wp, \
             tc.tile_pool(name="dstr", bufs=2) as wstr, \
             tc.tile_pool(name="dbig", bufs=1) as bigp, \
             tc.tile_pool(name="dpool", bufs=3) as sp, \
             tc.tile_pool(name="dps", bufs=2,
                          space=bass.MemorySpace.PSUM) as pp, \
             tc.tile_pool(name="dpsg", bufs=1,
                          space=bass.MemorySpace.PSUM) as ppg:
            ones = wp.tile([1, SB], DT_BF, tag="onesD")
            nc.vector.memset(ones[:], 1.0)
            chw_sb = load_w(wp, chw, "chw_sb")
            g1b_sb = load_w(wp, g1bias, "g1b_sb")
            aux_sb = load_w(wp, auxw, "aux_sb")
            sU = wp.tile([128, 1], DT_F32, tag="sU")
            sV = wp.tile([128, 1], DT_F32, tag="sV")
            nc.sync.dma_start(sU[:], selA[:])
            nc.sync.dma_start(sV[:], selB[:])
            hA = wp.tile([128, CH, B16], DT_BF, tag="hA")
            hB = wp.tile([128, CH, B16], DT_BF, tag="hB")
            cA = wp.tile([128, CH, B16], DT_F32, tag="cA")
            cB = wp.tile([128, CH, B16], DT_F32, tag="cB")
            nc.vector.memset(hA[:], 0.0)
            nc.vector.memset(cA[:], 0.0)
            hh, ccy = [hA, hB], [cA, cB]
            zblk = wp.tile([128, C2, SB], DT_BF, tag="zblk")
            nc.vector.memset(zblk[:], 0.0)

            for it in range(9):
                kb = min(it, 7)          # lstm_out block for the u-side
                # ---- G1SRC blend: u*lstm_out[kb] + v*AG(prev h1c1) ----
                g1 = bigp.tile([128, C2, SB], DT_BF, tag="g1src")
                uv = bigp.tile([128, C2, SB], DT_BF, tag="uvt")
                if it == 0:
                    vv = zblk
                else:
                    vv = bigp.tile([128, C2, SB], DT_BF, tag="vvt")
                    nc.sync.dma_start(
                        vv.rearrange("p a b -> p (a b)")[:],
                        AGO[it - 1][0].rearrange("p a b -> p (a b)")[:])
                for c in range(C2):
                    j, k = (0, c) if c < CH else (1, c - CH)
                    nc.sync.dma_start(
                        uv[:, c, :].rearrange("p (a b) -> p a b", a=32),
                        AGOUT[j, :, k, bass.ds(kb * 32, 32), :])
                for c in range(C2):
                    a1 = sp.tile([128, SB], DT_F32, tag="ba1")
                    nc.scalar.activation(a1[:], uv[:, c, :], AF.Copy,
                                         scale=sU[:])
                    a2 = sp.tile([128, SB], DT_F32, tag="ba2")
                    nc.scalar.activation(a2[:], vv[:, c, :], AF.Copy,
                                         scale=sV[:])
                    nc.vector.tensor_add(g1[:, c, :], a1[:], a2[:])
                # ---- GEMM1: xproj block = g1w.T @ g1 + bias ----
                xb = bigp.tile([128, GM, SB], DT_BF, tag="xbuf")
                for m in range(GM):
                    wt = wstr.tile([128, C2, 128], DT_BF, tag="g1wt")
                    for k in range(C2):
                        dma_eng().dma_start(wt[:, k, :],
                                            g1w[:, k, bass.ts(m, 128)])
                    ps = pp.tile([128, SB], DT_F32, tag="psD")
                    for k in range(C2):
                        nc.tensor.matmul(ps[:], wt[:, k, :], g1[:, k, :],
                                         start=(k == 0), stop=False)
                    nc.tensor.matmul(ps[:], g1b_sb[:, bass.ts(m, 128)],
                                     ones[:], start=False, stop=True)
                    nc.vector.tensor_copy(xb[:, m, :], ps[:])
                # ---- chain: 32 steps with data-driven carry masks ----
                hist = bigp.tile([128, C2, SB], DT_BF, tag="hist")
                mA = bigp.tile([128, CH, SB], DT_BF, tag="mA")
                mB = bigp.tile([128, CH, SB], DT_BF, tag="mB")
                nc.sync.dma_start(mA[:], maskA[:, :, bass.ds(it * SB, SB)])
                nc.sync.dma_start(mB[:], maskB[:, :, bass.ds(it * SB, SB)])
                for i in range(32):
                    step = it * 32 + i
                    hprev, hnext = hh[step % 2], hh[(step + 1) % 2]
                    cprev, cnext = ccy[step % 2], ccy[(step + 1) % 2]
                    psA = ppg.tile([128, 2 * CH, B16], DT_F32, tag="gDA")
                    psG = ppg.tile([128, CH, B16], DT_F32, tag="gDG")
                    psO = ppg.tile([128, CH, B16], DT_F32, tag="gDO")
                    for m in range(GM):
                        if m < 2 * CH:
                            dst = psA[:, m, :]
                        elif m < 3 * CH:
                            dst = psG[:, m - 2 * CH, :]
                        else:
                            dst = psO[:, m - 3 * CH, :]
                        for k in range(CH):
                            nc.tensor.matmul(
                                dst, chw_sb[:, k, bass.ts(m, 128)],
                                hprev[:, k, :],
                                start=(k == 0), stop=(k == CH - 1))
                    g = sp.tile([128, GM, B16], DT_F32, tag="gsD")
                    xbs = xb[:, :, i * B16:(i + 1) * B16]
                    nc.vector.tensor_add(
                        g[:, 0:2 * CH, :], psA[:], xbs[:, 0:2 * CH, :])
                    sif = sp.tile([128, 2 * CH, B16], DT_F32, tag="sifD")
                    nc.scalar.activation(sif[:], g[:, 0:2 * CH, :],
                                         AF.Sigmoid, scale=INVQ)
                    nc.vector.tensor_add(
                        g[:, 2 * CH:3 * CH, :], psG[:],
                        xbs[:, 2 * CH:3 * CH, :])
                    nc.vector.tensor_add(
                        g[:, 3 * CH:4 * CH, :], psO[:],
                        xbs[:, 3 * CH:4 * CH, :])
                    tg = sp.tile([128, CH, B16], DT_F32, tag="tgD")
                    nc.scalar.activation(tg[:], g[:, 2 * CH:3 * CH, :],
                                         AF.Tanh, scale=INVQ)
                    so = sp.tile([128, CH, B16], DT_F32, tag="soD")
                    nc.scalar.activation(so[:], g[:, 3 * CH:4 * CH, :],
                                         AF.Sigmoid, scale=INVQ)
                    t1 = sp.tile([128, CH, B16], DT_F32, tag="t1D")
                    nc.vector.tensor_mul(t1[:], sif[:, CH:2 * CH, :],
                                         cprev[:])
                    t2 = sp.tile([128, CH, B16], DT_F32, tag="t2D")
                    nc.vector.tensor_mul(t2[:], sif[:, 0:CH, :], tg[:])
                    cf = sp.tile([128, CH, B16], DT_F32, tag="cfD")
                    nc.vector.tensor_add(cf[:], t1[:], t2[:])
                    th = sp.tile([128, CH, B16], DT_F32, tag="thD")
                    nc.scalar.activation(th[:], cf[:], AF.Tanh)
                    hf = sp.tile([128, CH, B16], DT_F32, tag="hfD")
                    nc.vector.tensor_mul(hf[:], so[:], th[:])
                    nc.vector.tensor_copy(
                        hist[:, 0:CH, i * B16:(i + 1) * B16], hf[:])
                    nc.gpsimd.tensor_copy(
                        hist[:, CH:C2, i * B16:(i + 1) * B16], cf[:])
                    msA = mA[:, :, i * B16:(i + 1) * B16]
                    msB = mB[:, :, i * B16:(i + 1) * B16]
                    d1 = sp.tile([128, CH, B16], DT_F32, tag="d1D")
                    nc.vector.tensor_mul(d1[:], hf[:], msB)
                    d2 = sp.tile([128, CH, B16], DT_F32, tag="d2D")
                    nc.vector.tensor_mul(d2[:], hprev[:], msA)
                    nc.vector.tensor_add(hnext[:], d1[:], d2[:])
                    d3 = sp.tile([128, CH, B16], DT_F32, tag="d3D")
                    nc.vector.tensor_mul(d3[:], cf[:], msB)
                    d4 = sp.tile([128, CH, B16], DT_F32, tag="d4D")
                    nc.vector.tensor_mul(d4[:], cprev[:], msA)
                    nc.vector.tensor_add(cnext[:], d3[:], d4[:])
                # ---- aux: auxsrc = u*g1 + v*hist ; aux GEMM -> aux_t ----
                ax = bigp.tile([128, C2, SB], DT_BF, tag="axsrc")
                for c in range(C2):
                    a1 = sp.tile([128, SB], DT_F32, tag="axa1")
                    nc.scalar.activation(a1[:], g1[:, c, :], AF.Copy,
                                         scale=sU[:])
                    a2 = sp.tile([128, SB], DT_F32, tag="axa2")
                    nc.scalar.activation(a2[:], hist[:, c, :], AF.Copy,
                                         scale=sV[:])
                    nc.vector.tensor_add(ax[:, c, :], a1[:], a2[:])
                psx = pp.tile([2, SB], DT_F32, tag="psAux")
                for k in range(C2):
                    nc.tensor.matmul(psx[:], aux_sb[:, k, :], ax[:, k, :],
                                     start=(k == 0), stop=(k == C2 - 1))
                ox = sp.tile([2, SB], DT_F32, tag="oaux")
                nc.vector.tensor_copy(ox[:], psx[:])
                nc.sync.dma_start(aux_t[:, bass.ds(it * SB, SB)], ox[:])
                # ---- handoff AG (blocks 0-7 only) ----
                if it < 8:
                    nc.sync.dma_start(
                        AGB[it].rearrange("p a b -> p (a b)")[:],
                        hist.rearrange("p a b -> p (a b)")[:])
                    nc.gpsimd.collective_compute(
                        "AllGather", mybir.AluOpType.bypass,
                        replica_groups=RG,
                        ins=[AGB[it][:]],
                        outs=[AGO[it][:]],
                    )

    nc.compile()
    return nc


# --------------------------------------------------------------------------
# host side
# --------------------------------------------------------------------------

NIT, SB32 = 9, 32 * B16


def _wT_tiles(w, KD, scale=1.0, dtype=BF16):
    M, K = w.shape
    assert K == KD
    wt = np.ascontiguousarray(w.T).reshape(K // 128, 128, M)
    arr = np.ascontiguousarray(wt.transpose(1, 0, 2)).astype(np.float32)
    if scale != 1.0:
        arr = arr * scale
    if dtype is F8E3:
        arr = np.clip(arr, -15.5, 15.5)
    return arr.astype(dtype)


def _mask6(mask_tb, CH):
    S_, B_ = mask_tb.shape
    flat = mask_tb.reshape(-1)
    out = np.broadcast_to(flat[None, None, :], (128, CH, S_ * B_))
    return np.ascontiguousarray(out).astype(BF16)


def prepare_inputs6(inputs):
    CH = H // 128
    x = np.asarray(inputs["hidden_state"], np.float32)
    golds = np.asarray(inputs["golds"]).astype(np.int32)

    cls_W = np.asarray(inputs["cls_W"], np.float32)
    cls1 = _wT_tiles(cls_W[:, :H], H)                  # [128, 6, 2]
    aux_hi = np.concatenate([cls1, np.zeros_like(cls1)], axis=1)

    Q = QSCALE
    low_shared = dict(
        g1w=_wT_tiles(np.asarray(inputs["subw_Wih"], np.float32), 2 * H,
                      scale=Q),
        g1bias=(Q * np.asarray(inputs["subw_b"], np.float32))[None, :]
        .astype(BF16),
        chw=_wT_tiles(np.asarray(inputs["subw_Whh"], np.float32), H,
                      scale=Q, dtype=F8E3),
        auxw=_wT_tiles(cls_W[:, H:], 2 * H),
        wih=_wT_tiles(np.asarray(inputs["lstm_Wih_f"], np.float32), H,
                      scale=Q),
        whh=_wT_tiles(np.asarray(inputs["lstm_Whh_f"], np.float32), H,
                      scale=Q, dtype=F8E3),
        biasg=(Q * np.asarray(inputs["lstm_b_f"], np.float32))[None, :]
        .astype(BF16),
        selA=np.ones((128, 1), np.float32),
        selB=np.zeros((128, 1), np.float32),
    )
    high_shared = dict(
        g1w=_wT_tiles(np.asarray(inputs["word_Wih"], np.float32), 2 * H,
                      scale=Q),
        g1bias=(Q * np.asarray(inputs["word_b"], np.float32))[None, :]
        .astype(BF16),
        chw=_wT_tiles(np.asarray(inputs["word_Whh"], np.float32), H,
                      scale=Q, dtype=F8E3),
        auxw=aux_hi,
        wih=_wT_tiles(np.asarray(inputs["lstm_Wih_b"], np.float32), H,
                      scale=Q),
        whh=_wT_tiles(np.asarray(inputs["lstm_Whh_b"], np.float32), H,
                      scale=Q, dtype=F8E3),
        biasg=(Q * np.asarray(inputs["lstm_b_b"], np.float32))[None, :]
        .astype(BF16),
        selA=np.zeros((128, 1), np.float32),
        selB=np.ones((128, 1), np.float32),
    )

    NSTEP = NIT * 32
    in_maps = [None] * NCORES
    for p in range(4):
        xs = x[16 * p:16 * p + 16]
        xt = xs.transpose(2, 1, 0).reshape(CH, 128, S, B16)
        xT = np.ascontiguousarray(
            xt.transpose(1, 0, 2, 3).reshape(128, CH, S * B16)).astype(BF16)
        xTr = np.ascontiguousarray(
            xt[:, :, ::-1, :].transpose(1, 0, 2, 3).reshape(
                128, CH, S * B16)).astype(BF16)
        m = (golds[16 * p:16 * p + 16, 1:] > 0).astype(np.float32).T  # [255,16]
        z1 = np.zeros((1, B16), np.float32)
        # low (subword): A=0; B=keep for steps 0..254, 0 after
        loA = np.zeros((NSTEP, B16), np.float32)
        loB = np.concatenate([1.0 - m, np.zeros((NSTEP - 255, B16),
                                                np.float32)], 0)
        # high (word): block 0 holds; then A=1-sel, B=sel; pad holds
        hiA = np.concatenate([np.ones((32, B16), np.float32), 1.0 - m,
                              np.ones((NSTEP - 32 - 255, B16),
                                      np.float32)], 0)
        hiB = np.concatenate([np.zeros((32, B16), np.float32), m,
                              np.zeros((NSTEP - 32 - 255, B16),
                                       np.float32)], 0)
        low = dict(low_shared)
        low.update(xT16=xT, maskA=_mask6(loA, CH), maskB=_mask6(loB, CH))
        high = dict(high_shared)
        high.update(xT16=xTr, maskA=_mask6(hiA, CH), maskB=_mask6(hiB, CH))
        in_maps[p] = low
        in_maps[4 + p] = high

    assembly = dict(cls_b=np.asarray(inputs["cls_b"], np.float32))
    return in_maps, assembly


def assemble_output6(results, assembly):
    cls_b = assembly["cls_b"]
    out = np.empty((64, S, 2), np.float32)
    for p in range(4):
        cx = results[p]["aux_t"][:, 0:S * B16].reshape(2, S, B16)
        wc = results[4 + p]["aux_t"][:, SB32:SB32 + S * B16].reshape(
            2, S, B16)
        for j in range(2):
            out[16 * p:16 * p + 16, 1:, j] = (
                cx[j, 1:, :] + wc[j, :S - 1, :]).T + cls_b[j]
    out[:, 0, 0] = -1.0
    out[:, 0, 1] = 1.0
    return out


# --------------------------------------------------------------------------
# entry point
# --------------------------------------------------------------------------

FULL = dict(S=S, B=B8, H=H, NCORES=NCORES)

_CACHE = {}


def _get_program():
    if "p" not in _CACHE:
        _CACHE["p"] = build_program6()
    return _CACHE["p"]


def prepare_inputs(inputs, *_args):
    return prepare_inputs6(inputs)


def assemble_output(results, assembly):
    return assemble_output6(results, assembly)


def run(inputs, trace=False):
    nc = _get_program()
    in_maps, assembly = prepare_inputs6(inputs)
    res = run_bass_kernel_spmd(
        nc, in_maps, core_ids=list(range(NCORES)), trace=trace)
    out = assemble_output6(res.results, assembly)
    return out, res


def kernel(**inputs) -> np.ndarray:
    out, _ = run(inputs, trace=False)
    return out



# revision 15
# speedup vs baseline: 1.1688x; 1.1688x over previous
"""Trainium2 Bass kernel for nn_BertFreezeSegmentor (BiLSTM + stack-decoder).

Structure (validated against the reference, rel err ~1.6e-3):
  - Gold actions are in {0,1}, so the decoder "stacks" collapse into
    conditional carries, applied as data-driven hold/update masks.
  - All x-projections are hoisted out of the recurrences into GEMMs; each
    recurrence step only needs its h @ Whh.T matmul (LDWEIGHTS-bound).
  - Phase 1: core pairs (p, p+4) split each 16-example batch slice: low
    core runs the FWD scan, high core the BWD scan on host-reversed input
    (B=16 each). Histories merge via a pairwise AllGather.
  - Phase 2 (pipelined decode): the low core runs the subword chain for
    the pair's 16 examples; per 32-step block its (h1,c1) outputs are
    handed to the high core via a pairwise AllGather, where the word-Wih
    GEMM and the word chain run one block behind. The classifier GEMMs
    ride along per block (cx on low, wcls on high, one shared output).
  - Per-step gate adds are split per gate group so the i/f/g activations
    and the cell update overlap the o-gate m-tile matmuls.
  - The program is SPMD-uniform: all role divergence (weights, input
    order, carry masks, blend selects) is per-core input data.
"""

import numpy as np
import ml_dtypes

import concourse.bass as bass
import concourse.tile as tile
from concourse import bacc, mybir
from concourse.bass_utils import run_bass_kernel_spmd

BF16 = ml_dtypes.bfloat16
F8E3 = ml_dtypes.float8_e3m4
DT_BF = mybir.dt.bfloat16
DT_F8 = mybir.dt.float8e3
DT_F8H = mybir.dt.float8e4      # moving-operand h dtype (scale 1)
DT_F32 = mybir.dt.float32
AF = mybir.ActivationFunctionType

S, H, B16, B8 = 256, 768, 16, 8
NCORES = 8
QSCALE = 64.0            # fp8 weight scale: w_fp8 = w * QSCALE (e3m4)
INVQ = 1.0 / QSCALE      # folded into the gate activations' scale


def build_program6(num_devices=8, unroll=32):
    CH = H // 128            # 6
    GM = 4 * H // 128        # 24
    C2 = 2 * H // 128        # 12
    NC16 = S * B16           # 4096
    NC8 = S * B8             # 2048
    NB = 512
    NBLK16 = NC16 // NB      # 8
    NBLK8 = NC8 // NB        # 4

    nc = bacc.Bacc("TRN2", target_bir_lowering=False, debug=False,
                   enable_asserts=False, num_devices=num_devices)

    def inp(name, shape, dt):
        return nc.dram_tensor(name, shape, dt, kind="ExternalInput").ap()

    def scratch(name, shape, dt, **kw):
        return nc.dram_tensor(name, shape, dt, kind="Internal", **kw).ap()

    def outp(name, shape, dt):
        return nc.dram_tensor(name, shape, dt, kind="ExternalOutput").ap()

    # ---- inputs (per-core role data) ----
    xT16 = inp("xT16", [128, CH, NC16], DT_BF)       # straight / reversed
    wih = inp("wih", [128, CH, 4 * H], DT_BF)        # fwd / bwd (pre-scaled)
    whh = inp("whh", [128, CH, 4 * H], DT_F8)        # fp8e3, w*QSCALE
    biasg = inp("biasg", [1, 4 * H], DT_BF)
    selA = inp("selA", [128, 1], DT_F32)             # 1 on low cores, 0 high
    selB = inp("selB", [128, 1], DT_F32)             # complement
    g1w = inp("g1w", [128, C2, 4 * H], DT_BF)       # swih / wwih (pre-scaled)
    g1bias = inp("g1bias", [1, 4 * H], DT_BF)        # sbias / wbias (pre-scaled)
    chw = inp("chw", [128, CH, 4 * H], DT_F8)        # fp8e3, w*QSCALE
    auxw = inp("auxw", [128, C2, 2], DT_BF)          # cls2T / [cls1T;0]
    NIT = 9                                          # pipeline iterations
    SB = 32 * B16                                    # block cols (512)
    maskA = inp("maskA", [128, CH, NIT * SB], DT_BF)
    maskB = inp("maskB", [128, CH, NIT * SB], DT_BF)

    # ---- scratch ----
    XS = scratch("XS", [128, GM, NC16], DT_BF)
    histS = scratch("histS", [128, CH, NC16], DT_BF)
    histR = scratch("histR", [128, CH, NC16], DT_BF)
    canonT = scratch("canonT", [128, CH, NC16], DT_BF)
    AGOUT = scratch("AGOUT", [2, 128, CH, S, B16], DT_BF)
    AGB = [scratch(f"AGB{k}", [128, C2, SB], DT_BF) for k in range(8)]
    AGO = [scratch(f"AGO{k}", [2, 128, C2, SB], DT_BF) for k in range(8)]

    # ---- outputs ----
    aux_t = outp("aux_t", [2, NIT * SB], DT_F32)

    RG = [[p, p + 4] for p in range(4)]

    with tile.TileContext(nc) as tc:

        _dma_rr = [0]

        def dma_eng():
            _dma_rr[0] += 1
            return nc.sync if _dma_rr[0] % 2 else nc.gpsimd

        def load_w(pool, src, tag):
            t = pool.tile(list(src.shape), src.dtype, tag=tag)
            if len(src.shape) == 3 and src.shape[1] > 1:
                for k in range(src.shape[1]):
                    dma_eng().dma_start(t[:, k, :], src[:, k, :])
            else:
                dma_eng().dma_start(t[:], src[:])
            return t

        # ============ Phase A2: XS = wih @ x + bias  (B=16) ============
        with tc.tile_pool(name="wA", bufs=1) as wp, \
             tc.tile_pool(name="gA", bufs=3) as pool, \
             tc.tile_pool(name="gA_ps", bufs=2,
                          space=bass.MemorySpace.PSUM) as psp:
            ones = wp.tile([1, NB], DT_BF, tag="ones")
            nc.vector.memset(ones[:], 1.0)
            wih_sb = load_w(wp, wih, "wih_sb")
            bia_sb = load_w(wp, biasg, "bia_sb")
            for nb in range(NBLK16):
                mv = pool.tile([128, CH, NB], DT_BF, tag="mvA")
                for k in range(CH):
                    dma_eng().dma_start(mv[:, k, :],
                                        xT16[:, k, bass.ts(nb, NB)])
                for m in range(GM):
                    ps = psp.tile([128, NB], DT_F32, tag="ps")
                    for k in range(CH):
                        nc.tensor.matmul(
                            ps[:], wih_sb[:, k, bass.ts(m, 128)], mv[:, k, :],
                            start=(k == 0), stop=False)
                    nc.tensor.matmul(ps[:], bia_sb[:, bass.ts(m, 128)],
                                     ones[:], start=False, stop=True)
                    ot = pool.tile([128, NB], DT_BF, tag="gout")
                    nc.vector.tensor_copy(ot[:], ps[:])
                    dma_eng().dma_start(XS[:, m, bass.ts(nb, NB)], ot[:])

        # ============ Phase B2: one scan, B=16, dual history write ========
        with tc.tile_pool(name="w_scan", bufs=1) as wp, \
             tc.tile_pool(name="scan", bufs=3) as sp, \
             tc.tile_pool(name="scan_ps", bufs=2,
                          space=bass.MemorySpace.PSUM) as pp:
            whh_sb = load_w(wp, whh, "whh_sb")
            c0 = wp.tile([128, CH, B16], DT_F32, tag="c0")
            c1 = wp.tile([128, CH, B16], DT_F32, tag="c1")
            h0 = wp.tile([128, CH, B16], DT_F8H, tag="h0")
            h1 = wp.tile([128, CH, B16], DT_F8H, tag="h1")
            nc.vector.memset(c0[:], 0.0)
            nc.vector.memset(h0[:], 0.0)
            cc, hh = [c0, c1], [h0, h1]

            def blk(iv0, cnt):
                xf = sp.tile([128, GM, unroll * B16], DT_BF, tag="xf")
                nc.sync.dma_start(
                    xf[:, :, 0:cnt * B16],
                    XS[:, :, bass.ds(iv0 * B16, cnt * B16)])
                for i in range(cnt):
                    t = iv0 + i
                    cprev, cnew = cc[i % 2], cc[(i + 1) % 2]
                    hprev, hnext = hh[i % 2], hh[(i + 1) % 2]
                    psA = pp.tile([128, 2 * CH, B16], DT_F32, tag="gA")
                    psG = pp.tile([128, CH, B16], DT_F32, tag="gG")
                    psO = pp.tile([128, CH, B16], DT_F32, tag="gO")
                    for m in range(GM):
                        if m < 2 * CH:
                            dst = psA[:, m, :]
                        elif m < 3 * CH:
                            dst = psG[:, m - 2 * CH, :]
                        else:
                            dst = psO[:, m - 3 * CH, :]
                        for k in range(CH):
                            nc.tensor.matmul(
                                dst, whh_sb[:, k, bass.ts(m, 128)],
                                hprev[:, k, :],
                                start=(k == 0), stop=(k == CH - 1))
                    g = sp.tile([128, GM, B16], DT_F32, tag="gs")
                    xfs = xf[:, :, i * B16:(i + 1) * B16]
                    nc.vector.tensor_add(
                        g[:, 0:2 * CH, :], psA[:], xfs[:, 0:2 * CH, :])
                    sif = sp.tile([128, 2 * CH, B16], DT_F32, tag="sif")
                    nc.scalar.activation(sif[:], g[:, 0:2 * CH, :],
                                         AF.Sigmoid, scale=INVQ)
                    nc.vector.tensor_add(
                        g[:, 2 * CH:3 * CH, :], psG[:],
                        xfs[:, 2 * CH:3 * CH, :])
                    nc.vector.tensor_add(
                        g[:, 3 * CH:4 * CH, :], psO[:],
                        xfs[:, 3 * CH:4 * CH, :])
                    tg = sp.tile([128, CH, B16], DT_F32, tag="tg")
                    nc.scalar.activation(tg[:], g[:, 2 * CH:3 * CH, :],
                                         AF.Tanh, scale=INVQ)
                    so = sp.tile([128, CH, B16], DT_F32, tag="so")
                    nc.scalar.activation(so[:], g[:, 3 * CH:4 * CH, :],
                                         AF.Sigmoid, scale=INVQ)
                    t1 = sp.tile([128, CH, B16], DT_F32, tag="t1")
                    nc.vector.tensor_mul(t1[:], sif[:, CH:2 * CH, :], cprev[:])
                    t2 = sp.tile([128, CH, B16], DT_F32, tag="t2")
                    nc.vector.tensor_mul(t2[:], sif[:, 0:CH, :], tg[:])
                    nc.vector.tensor_add(cnew[:], t1[:], t2[:])
                    th = sp.tile([128, CH, B16], DT_F32, tag="th")
                    nc.scalar.activation(th[:], cnew[:], AF.Tanh)
                    nc.vector.tensor_mul(hnext[:], so[:], th[:])
                    hbf = sp.tile([128, CH, B16], DT_BF, tag="hbf")
                    nc.vector.tensor_mul(hbf[:], so[:], th[:])
                    dma_eng().dma_start(
                        histS[:, :, bass.ds(t * B16, B16)], hbf[:])
                    dma_eng().dma_start(
                        histR[:, :, bass.ds((S - 1 - t) * B16, B16)],
                        hbf[:])

            tc.For_i_unrolled_general(0, S, 1, blk, max_unroll=unroll,
                                      hint_engines=(mybir.EngineType.PE,))

        # ============ blend canonT + pairwise AllGather ============
        with tc.tile_pool(name="blend", bufs=2) as bp:
            sA = bp.tile([128, 1], DT_F32, tag="sA")
            sB = bp.tile([128, 1], DT_F32, tag="sB")
            nc.sync.dma_start(sA[:], selA[:])
            nc.sync.dma_start(sB[:], selB[:])
            for k in range(CH):
                hs = bp.tile([128, NC16], DT_BF, tag="hs")
                hr = bp.tile([128, NC16], DT_BF, tag="hr")
                nc.sync.dma_start(hs[:], histS[:, k, :])
                nc.gpsimd.dma_start(hr[:], histR[:, k, :])
                a1 = bp.tile([128, NC16], DT_F32, tag="a1")
                nc.scalar.activation(a1[:], hs[:], AF.Copy, scale=sA[:])
                a2 = bp.tile([128, NC16], DT_F32, tag="a2")
                nc.scalar.activation(a2[:], hr[:], AF.Copy, scale=sB[:])
                cn = bp.tile([128, NC16], DT_BF, tag="cn")
                nc.vector.tensor_add(cn[:], a1[:], a2[:])
                nc.sync.dma_start(canonT[:, k, :], cn[:])

        nc.gpsimd.collective_compute(
            "AllGather", mybir.AluOpType.bypass,
            replica_groups=RG,
            ins=[canonT[:]],
            outs=[AGOUT[:]],
        )

        # ============ pipelined decode: sub (low) / word (high) ============
        # 9 iterations; low runs subword blocks 0-7 at iters 0-7, high runs
        # word blocks 0-7 at iters 1-8 (one-block lag). Handoff of h1/c1 is
        # a per-block pairwise AllGather; high's GEMM1 (wwih) consumes the
        # previous iteration's AG output, low's GEMM1 (swih) consumes the
        # lstm_out block from the big AG. All role divergence is input data.
        with tc.tile_pool(name="wD", bufs=1) as wp, \
             tc.tile_pool(name="dstr", bufs=2) as wstr, \
             tc.tile_pool(name="dbig", bufs=1) as bigp, \
             tc.tile_pool(name="dpool", bufs=3) as sp, \
             tc.tile_pool(name="dps", bufs=2,
                          space=bass.MemorySpace.PSUM) as pp, \
             tc.tile_pool(name="dpsg", bufs=1,
                          space=bass.MemorySpace.PSUM) as ppg:
            ones = wp.tile([1, SB], DT_BF, tag="onesD")
            nc.vector.memset(ones[:], 1.0)
            chw_sb = load_w(wp, chw, "chw_sb")
            g1b_sb = load_w(wp, g1bias, "g1b_sb")
            aux_sb = load_w(wp, auxw, "aux_sb")
            sU = wp.tile([128, 1], DT_F32, tag="sU")
            sV = wp.tile([128, 1], DT_F32, tag="sV")
            nc.sync.dma_start(sU[:], selA[:])
            nc.sync.dma_start(sV[:], selB[:])
            hA = wp.tile([128, CH, B16], DT_F8H, tag="hA")
            hB = wp.tile([128, CH, B16], DT_F8H, tag="hB")
            cA = wp.tile([128, CH, B16], DT_F32, tag="cA")
            cB = wp.tile([128, CH, B16], DT_F32, tag="cB")
            nc.vector.memset(hA[:], 0.0)
            nc.vector.memset(cA[:], 0.0)
            hh, ccy = [hA, hB], [cA, cB]
            zblk = wp.tile([128, C2, SB], DT_BF, tag="zblk")
            nc.vector.memset(zblk[:], 0.0)

            for it in range(9):
                kb = min(it, 7)          # lstm_out block for the u-side
                # ---- G1SRC blend: u*lstm_out[kb] + v*AG(prev h1c1) ----
                g1 = bigp.tile([128, C2, SB], DT_BF, tag="g1src")
                uv = bigp.tile([128, C2, SB], DT_BF, tag="uvt")
                if it == 0:
                    vv = zblk
                else:
                    vv = bigp.tile([128, C2, SB], DT_BF, tag="vvt")
                    nc.sync.dma_start(
                        vv.rearrange("p a b -> p (a b)")[:],
                        AGO[it - 1][0].rearrange("p a b -> p (a b)")[:])
                for c in range(C2):
                    j, k = (0, c) if c < CH else (1, c - CH)
                    nc.sync.dma_start(
                        uv[:, c, :].rearrange("p (a b) -> p a b", a=32),
                        AGOUT[j, :, k, bass.ds(kb * 32, 32), :])
                for c in range(C2):
                    a1 = sp.tile([128, SB], DT_F32, tag="ba1")
                    nc.scalar.activation(a1[:], uv[:, c, :], AF.Copy,
                                         scale=sU[:])
                    a2 = sp.tile([128, SB], DT_F32, tag="ba2")
                    nc.scalar.activation(a2[:], vv[:, c, :], AF.Copy,
                                         scale=sV[:])
                    nc.vector.tensor_add(g1[:, c, :], a1[:], a2[:])
                # ---- GEMM1: xproj block = g1w.T @ g1 + bias ----
                xb = bigp.tile([128, GM, SB], DT_BF, tag="xbuf")
                for m in range(GM):
                    wt = wstr.tile([128, C2, 128], DT_BF, tag="g1wt")
                    for k in range(C2):
                        dma_eng().dma_start(wt[:, k, :],
                                            g1w[:, k, bass.ts(m, 128)])
                    ps = pp.tile([128, SB], DT_F32, tag="psD")
                    for k in range(C2):
                        nc.tensor.matmul(ps[:], wt[:, k, :], g1[:, k, :],
                                         start=(k == 0), stop=False)
                    nc.tensor.matmul(ps[:], g1b_sb[:, bass.ts(m, 128)],
                                     ones[:], start=False, stop=True)
                    nc.vector.tensor_copy(xb[:, m, :], ps[:])
                # ---- chain: 32 steps with data-driven carry masks ----
                hist = bigp.tile([128, C2, SB], DT_BF, tag="hist")
                mA = bigp.tile([128, CH, SB], DT_BF, tag="mA")
                mB = bigp.tile([128, CH, SB], DT_BF, tag="mB")
                nc.sync.dma_start(mA[:], maskA[:, :, bass.ds(it * SB, SB)])
                nc.sync.dma_start(mB[:], maskB[:, :, bass.ds(it * SB, SB)])
                for i in range(32):
                    step = it * 32 + i
                    hprev, hnext = hh[step % 2], hh[(step + 1) % 2]
                    cprev, cnext = ccy[step % 2], ccy[(step + 1) % 2]
                    psA = ppg.tile([128, 2 * CH, B16], DT_F32, tag="gDA")
                    psG = ppg.tile([128, CH, B16], DT_F32, tag="gDG")
                    psO = ppg.tile([128, CH, B16], DT_F32, tag="gDO")
                    for m in range(GM):
                        if m < 2 * CH:
                            dst = psA[:, m, :]
                        elif m < 3 * CH:
                            dst = psG[:, m - 2 * CH, :]
                        else:
                            dst = psO[:, m - 3 * CH, :]
                        for k in range(CH):
                            nc.tensor.matmul(
                                dst, chw_sb[:, k, bass.ts(m, 128)],
                                hprev[:, k, :],
                                start=(k == 0), stop=(k == CH - 1))
                    g = sp.tile([128, GM, B16], DT_F32, tag="gsD")
                    xbs = xb[:, :, i * B16:(i + 1) * B16]
                    nc.vector.tensor_add(
                        g[:, 0:2 * CH, :], psA[:], xbs[:, 0:2 * CH, :])
                    sif = sp.tile([128, 2 * CH, B16], DT_F32, tag="sifD")
                    nc.scalar.activation(sif[:], g[:, 0:2 * CH, :],
                                         AF.Sigmoid, scale=INVQ)
                    nc.vector.tensor_add(
                        g[:, 2 * CH:3 * CH, :], psG[:],
                        xbs[:, 2 * CH:3 * CH, :])
                    nc.vector.tensor_add(
                        g[:, 3 * CH:4 * CH, :], psO[:],
                        xbs[:, 3 * CH:4 * CH, :])
                    tg = sp.tile([128, CH, B16], DT_F32, tag="tgD")
                    nc.scalar.activation(tg[:], g[:, 2 * CH:3 * CH, :],
                                         AF.Tanh, scale=INVQ)
                    so = sp.tile([128, CH, B16], DT_F32, tag="soD")
                    nc.scalar.activation(so[:], g[:, 3 * CH:4 * CH, :],
                                         AF.Sigmoid, scale=INVQ)
                    t1 = sp.tile([128, CH, B16], DT_F32, tag="t1D")
                    nc.vector.tensor_mul(t1[:], sif[:, CH:2 * CH, :],
                                         cprev[:])
                    t2 = sp.tile([128, CH, B16], DT_F32, tag="t2D")
                    nc.vector.tensor_mul(t2[:], sif[:, 0:CH, :], tg[:])
                    cf = sp.tile([128, CH, B16], DT_F32, tag="cfD")
                    nc.vector.tensor_add(cf[:], t1[:], t2[:])
                    th = sp.tile([128, CH, B16], DT_F32, tag="thD")
                    nc.scalar.activation(th[:], cf[:], AF.Tanh)
                    hf = sp.tile([128, CH, B16], DT_F32, tag="hfD")
                    nc.vector.tensor_mul(hf[:], so[:], th[:])
                    nc.vector.tensor_copy(
                        hist[:, 0:CH, i * B16:(i + 1) * B16], hf[:])
                    nc.gpsimd.tensor_copy(
                        hist[:, CH:C2, i * B16:(i + 1) * B16], cf[:])
                    msA = mA[:, :, i * B16:(i + 1) * B16]
                    msB = mB[:, :, i * B16:(i + 1) * B16]
                    d1 = sp.tile([128, CH, B16], DT_F32, tag="d1D")
                    nc.vector.tensor_mul(d1[:], hf[:], msB)
                    d2 = sp.tile([128, CH, B16], DT_F32, tag="d2D")
                    nc.vector.tensor_mul(d2[:], hprev[:], msA)
                    nc.vector.tensor_add(hnext[:], d1[:], d2[:])
                    d3 = sp.tile([128, CH, B16], DT_F32, tag="d3D")
                    nc.vector.tensor_mul(d3[:], cf[:], msB)
                    d4 = sp.tile([128, CH, B16], DT_F32, tag="d4D")
                    nc.vector.tensor_mul(d4[:], cprev[:], msA)
                    nc.vector.tensor_add(cnext[:], d3[:], d4[:])
                # ---- aux: auxsrc = u*g1 + v*hist ; aux GEMM -> aux_t ----
                ax = bigp.tile([128, C2, SB], DT_BF, tag="axsrc")
                for c in range(C2):
                    a1 = sp.tile([128, SB], DT_F32, tag="axa1")
                    nc.scalar.activation(a1[:], g1[:, c, :], AF.Copy,
                                         scale=sU[:])
                    a2 = sp.tile([128, SB], DT_F32, tag="axa2")
                    nc.scalar.activation(a2[:], hist[:, c, :], AF.Copy,
                                         scale=sV[:])
                    nc.vector.tensor_add(ax[:, c, :], a1[:], a2[:])
                psx = pp.tile([2, SB], DT_F32, tag="psAux")
                for k in range(C2):
                    nc.tensor.matmul(psx[:], aux_sb[:, k, :], ax[:, k, :],
                                     start=(k == 0), stop=(k == C2 - 1))
                ox = sp.tile([2, SB], DT_F32, tag="oaux")
                nc.vector.tensor_copy(ox[:], psx[:])
                nc.sync.dma_start(aux_t[:, bass.ds(it * SB, SB)], ox[:])
                # ---- handoff AG (blocks 0-7 only) ----
                if it < 8:
                    nc.sync.dma_start(
                        AGB[it].rearrange("p a b -> p (a b)")[:],
                        hist.rearrange("p a b -> p (a b)")[:])
                    nc.gpsimd.collective_compute(
                        "AllGather", mybir.AluOpType.bypass,
                        replica_groups=RG,
                        ins=[AGB[it][:]],
                        outs=[AGO[it][:]],
                    )

    nc.compile()
    return nc


# --------------------------------------------------------------------------
# host side
# --------------------------------------------------------------------------

NIT, SB32 = 9, 32 * B16


def _wT_tiles(w, KD, scale=1.0, dtype=BF16):
    M, K = w.shape
    assert K == KD
    wt = np.ascontiguousarray(w.T).reshape(K // 128, 128, M)
    arr = np.ascontiguousarray(wt.transpose(1, 0, 2)).astype(np.float32)
    if scale != 1.0:
        arr = arr * scale
    if dtype is F8E3:
        arr = np.clip(arr, -15.5, 15.5)
    return arr.astype(dtype)


def _mask6(mask_tb, CH):
    S_, B_ = mask_tb.shape
    flat = mask_tb.reshape(-1)
    out = np.broadcast_to(flat[None, None, :], (128, CH, S_ * B_))
    return np.ascontiguousarray(out).astype(BF16)


def prepare_inputs6(inputs):
    CH = H // 128
    x = np.asarray(inputs["hidden_state"], np.float32)
    golds = np.asarray(inputs["golds"]).astype(np.int32)

    cls_W = np.asarray(inputs["cls_W"], np.float32)
    cls1 = _wT_tiles(cls_W[:, :H], H)                  # [128, 6, 2]
    aux_hi = np.concatenate([cls1, np.zeros_like(cls1)], axis=1)

    Q = QSCALE
    low_shared = dict(
        g1w=_wT_tiles(np.asarray(inputs["subw_Wih"], np.float32), 2 * H,
                      scale=Q),
        g1bias=(Q * np.asarray(inputs["subw_b"], np.float32))[None, :]
        .astype(BF16),
        chw=_wT_tiles(np.asarray(inputs["subw_Whh"], np.float32), H,
                      scale=Q, dtype=F8E3),
        auxw=_wT_tiles(cls_W[:, H:], 2 * H),
        wih=_wT_tiles(np.asarray(inputs["lstm_Wih_f"], np.float32), H,
                      scale=Q),
        whh=_wT_tiles(np.asarray(inputs["lstm_Whh_f"], np.float32), H,
                      scale=Q, dtype=F8E3),
        biasg=(Q * np.asarray(inputs["lstm_b_f"], np.float32))[None, :]
        .astype(BF16),
        selA=np.ones((128, 1), np.float32),
        selB=np.zeros((128, 1), np.float32),
    )
    high_shared = dict(
        g1w=_wT_tiles(np.asarray(inputs["word_Wih"], np.float32), 2 * H,
                      scale=Q),
        g1bias=(Q * np.asarray(inputs["word_b"], np.float32))[None, :]
        .astype(BF16),
        chw=_wT_tiles(np.asarray(inputs["word_Whh"], np.float32), H,
                      scale=Q, dtype=F8E3),
        auxw=aux_hi,
        wih=_wT_tiles(np.asarray(inputs["lstm_Wih_b"], np.float32), H,
                      scale=Q),
        whh=_wT_tiles(np.asarray(inputs["lstm_Whh_b"], np.float32), H,
                      scale=Q, dtype=F8E3),
        biasg=(Q * np.asarray(inputs["lstm_b_b"], np.float32))[None, :]
        .astype(BF16),
        selA=np.zeros((128, 1), np.float32),
        selB=np.ones((128, 1), np.float32),
    )

    NSTEP = NIT * 32
    in_maps = [None] * NCORES
    for p in range(4):
        xs = x[16 * p:16 * p + 16]
        xt = xs.transpose(2, 1, 0).reshape(CH, 128, S, B16)
        xT = np.ascontiguousarray(
            xt.transpose(1, 0, 2, 3).reshape(128, CH, S * B16)).astype(BF16)
        xTr = np.ascontiguousarray(
            xt[:, :, ::-1, :].transpose(1, 0, 2, 3).reshape(
                128, CH, S * B16)).astype(BF16)
        m = (golds[16 * p:16 * p + 16, 1:] > 0).astype(np.float32).T  # [255,16]
        z1 = np.zeros((1, B16), np.float32)
        # low (subword): A=0; B=keep for steps 0..254, 0 after
        loA = np.zeros((NSTEP, B16), np.float32)
        loB = np.concatenate([1.0 - m, np.zeros((NSTEP - 255, B16),
                                                np.float32)], 0)
        # high (word): block 0 holds; then A=1-sel, B=sel; pad holds
        hiA = np.concatenate([np.ones((32, B16), np.float32), 1.0 - m,
                              np.ones((NSTEP - 32 - 255, B16),
                                      np.float32)], 0)
        hiB = np.concatenate([np.zeros((32, B16), np.float32), m,
                              np.zeros((NSTEP - 32 - 255, B16),
                                       np.float32)], 0)
        low = dict(low_shared)
        low.update(xT16=xT, maskA=_mask6(loA, CH), maskB=_mask6(loB, CH))
        high = dict(high_shared)
        high.update(xT16=xTr, maskA=_mask6(hiA, CH), maskB=_mask6(hiB, CH))
        in_maps[p] = low
        in_maps[4 + p] = high

    assembly = dict(cls_b=np.asarray(inputs["cls_b"], np.float32))
    return in_maps, assembly


def assemble_output6(results, assembly):
    cls_b = assembly["cls_b"]
    out = np.empty((64, S, 2), np.float32)
    for p in range(4):
        cx = results[p]["aux_t"][:, 0:S * B16].reshape(2, S, B16)
        wc = results[4 + p]["aux_t"][:, SB32:SB32 + S * B16].reshape(
            2, S, B16)
        for j in range(2):
            out[16 * p:16 * p + 16, 1:, j] = (
                cx[j, 1:, :] + wc[j, :S - 1, :]).T + cls_b[j]
    out[:, 0, 0] = -1.0
    out[:, 0, 1] = 1.0
    return out


# --------------------------------------------------------------------------
# entry point
# --------------------------------------------------------------------------

FULL = dict(S=S, B=B8, H=H, NCORES=NCORES)

_CACHE = {}


def _get_program():
    if "p" not in _CACHE:
        _CACHE["p"] = build_program6()
    return _CACHE["p"]


def prepare_inputs(inputs, *_args):
    return prepare_inputs6(inputs)


def assemble_output(results, assembly):
    return assemble_output6(results, assembly)


def run(inputs, trace=False):
    nc = _get_program()
    in_maps, assembly = prepare_inputs6(inputs)
    res = run_bass_kernel_spmd(
        nc, in_maps, core_ids=list(range(NCORES)), trace=trace)
    out = assemble_output6(res.results, assembly)
    return out, res


def kernel(**inputs) -> np.ndarray:
    out, _ = run(inputs, trace=False)
    return out

